# revision 10
# baseline (speedup 1.0000x reference)
"""Causal self-attention (GQA, rope, qk-rmsnorm) Trainium2 kernel, 8 NeuronCores.

Sharding: core = (b, g), b = core // 4 (batch), g = core % 4.
Each core handles query row-chunks {g, 4+g, 8+g, 12+g} (128 rows each) of its
batch: computes Q for those 512 rows, K/V for its OWN 512 rows only (the four
cores of a batch then AllGather K/V), attention for all 16 heads over its 512
query rows, and its 512-row slice of the output projection. Host gathers row
slices. The program is identical on all cores (SPMD); all per-core variation
comes through the input shards.

Host pre-transposes x (bf16) so no PE transposes of x are needed on device.
Weights are host-converted to bf16. Causal masking is a 0/1 DVE multiply on
the exp'd diagonal block (host-provided per-core mask); the softmax
denominator is computed with one PE reduction per 4 key tiles (vector
pre-sum). Q-projection slabs are interleaved with attention head groups so
the scalar-engine exp stream overlaps PE matmuls.
"""

import sys

if "/opt/trn_rl_repo" not in sys.path:
    sys.path.insert(0, "/opt/trn_rl_repo")

import numpy as np

B, T, C = 2, 2048, 2048
NH, NKV = 16, 4
HD = C // NH  # 128
P = 128
NT = T // P            # 16 token tiles per batch
NCT = C // P           # 16 contraction tiles
QROWS = 512            # own query rows per core
NQT = QROWS // P       # 4 own token tiles
SCALE = 1.0 / float(np.sqrt(HD))
EPS = float(np.finfo(np.float32).eps)

_CACHE = {}


def _chunks(g):
    return [g, 4 + g, 8 + g, 12 + g]


def _rows(g):
    return np.concatenate([np.arange(ch * P, (ch + 1) * P) for ch in _chunks(g)])


def _qmask01_t(g):
    """0/1 multiplicative mask, transposed layout: (slot c, sub s, k i, q j).

    For slot c the score tile is S^T[k, q] with k in [0, 512*(c+1)) and q the
    128 rows of chunk 4c+g. Only keys in the last 512 of the slot can be
    invalid; mask[c, s, i, j] = 1 if key (512*c + s*128 + i) <= query
    (128*(4c+g) + j) else 0.
    """
    m = np.zeros((4, 4, P, P), np.float32)
    for c in range(4):
        k0 = 512 * c
        r0 = (4 * c + g) * P
        k = k0 + np.arange(512)[:, None]          # (512, 1)
        q = r0 + np.arange(P)[None, :]            # (1, 128)
        m[c] = np.where(k <= q, 1.0, 0.0).reshape(4, P, P)
    return m


def _build():
    import concourse.bacc as bacc
    import concourse.bass as bass
    import concourse.mybir as mybir
    import concourse.tile as tile
    from concourse.masks import make_identity

    f32 = mybir.dt.float32
    bf16 = mybir.dt.bfloat16
    AF = mybir.ActivationFunctionType
    OP = mybir.AluOpType
    AX = mybir.AxisListType

    nc = bacc.Bacc("TRN2", target_bir_lowering=False, debug=False, num_devices=8)

    xoT = nc.dram_tensor("xoT", [NCT, P, QROWS], bf16, kind="ExternalInput").ap()
    coso = nc.dram_tensor("coso", [QROWS, HD // 2], f32, kind="ExternalInput").ap()
    sino = nc.dram_tensor("sino", [QROWS, HD // 2], f32, kind="ExternalInput").ap()
    wq = nc.dram_tensor("wq", [C, C], bf16, kind="ExternalInput").ap()
    wk = nc.dram_tensor("wk", [C, NKV * HD], bf16, kind="ExternalInput").ap()
    wv = nc.dram_tensor("wv", [C, NKV * HD], bf16, kind="ExternalInput").ap()
    wo = nc.dram_tensor("wo", [C, C], bf16, kind="ExternalInput").ap()
    qm = nc.dram_tensor("qm", [4, 4, P, P], bf16, kind="ExternalInput").ap()
    yo = nc.dram_tensor("yo", [QROWS, C], f32, kind="ExternalOutput").ap()

    def bcast4(ap2d):
        # [128, 64] -> [128, 4, 64] with middle step 0 (replicate across heads)
        return bass.AP(
            tensor=ap2d.tensor,
            offset=ap2d.offset,
            ap=[ap2d.ap[0], [0, 4], ap2d.ap[1]],
        )

    with tile.TileContext(nc) as tc:
        with (
            tc.tile_pool(name="singles", bufs=1) as singles,
            tc.tile_pool(name="big", bufs=1) as bigpool,
            tc.tile_pool(name="wsl", bufs=2) as wslpool,
            tc.tile_pool(name="epi", bufs=2) as epipool,
            tc.tile_pool(name="qh", bufs=3) as qhpool,
            tc.tile_pool(name="wo3", bufs=2) as wopool,
            tc.tile_pool(name="pt", bufs=4) as ptpool,
            tc.tile_pool(name="pts", bufs=2) as ptspool,
            tc.tile_pool(name="smallf", bufs=2) as smallf,
            tc.tile_pool(name="outs", bufs=2) as outpool,
            tc.tile_pool(name="dram", bufs=1, space="DRAM") as drampool,
            tc.tile_pool(name="psS", bufs=3, space="PSUM") as psS,
            tc.tile_pool(name="psY", bufs=3, space="PSUM") as psY,
            tc.tile_pool(name="psD", bufs=2, space="PSUM") as psD,
        ):
            ident = singles.tile([P, P], bf16)
            make_identity(nc, ident)
            ones128 = singles.tile([P, P], bf16)
            nc.vector.memset(ones128, 1.0)
            eps_q = singles.tile([P, 1], f32)
            nc.vector.memset(eps_q, EPS)
            eps_k = singles.tile([P, 1], f32)
            nc.vector.memset(eps_k, HD * EPS)

            # persistent big SBUF tensors
            xT = bigpool.tile([P, NCT, QROWS], bf16, tag="xT")     # [c, ct, q]
            qT = bigpool.tile([P, NH, QROWS], bf16, tag="qT")      # [d, h, q]
            kT = bigpool.tile([P, NKV, T], bf16, tag="kT")         # [d, kvh, k]
            vA = bigpool.tile([P, NT, NKV, HD], bf16, tag="vA")    # [ktok, tt, kvh, d]
            yT = bigpool.tile([P, NCT, QROWS], bf16, tag="yT")     # [d, ct, q]
            qmask = singles.tile([P, 4, 4, P], bf16)               # [ki, c, sub, q]
            nc.scalar.dma_start(out=qmask, in_=qm.rearrange("c s i j -> i c s j"))

            # x^T load: one DMA
            nc.gpsimd.dma_start(out=xT, in_=xoT.rearrange("a p n -> p a n"))

            # cos/sin tiles for the 4 own token tiles, persistent, loaded once
            cos4 = [singles.tile([P, 4, 64], f32, name=f"cos4_{t}") for t in range(NQT)]
            sin4 = [singles.tile([P, 4, 64], f32, name=f"sin4_{t}") for t in range(NQT)]
            for t in range(NQT):
                nc.scalar.dma_start(out=cos4[t], in_=bcast4(coso[t * P:(t + 1) * P, :]))
                nc.scalar.dma_start(out=sin4[t], in_=bcast4(sino[t * P:(t + 1) * P, :]))

            # collective bounce buffers (DRAM)
            cc_inK = drampool.tile([NQT, P, 512], bf16)
            cc_outK = drampool.tile([4, NQT, P, 512], bf16)
            cc_inV = drampool.tile([NQT, P, 512], bf16)
            cc_outV = drampool.tile([4, NQT, P, 512], bf16)

            # ---------------- helpers ----------------
            def load_w_slab(w_ap, col0, name):
                """One 512-col slab of a weight, as [128, 16, 512] bf16."""
                wsl = wslpool.tile([P, NCT, 512], bf16, tag="wsl", name=name)
                nc.sync.dma_start(
                    out=wsl,
                    in_=w_ap[:, col0:col0 + 512].rearrange("(a p) n -> p a n", p=P),
                )
                return wsl

            def rope_rms(ps, c4, s4, out_bf, eps_ap, sqrt_scale):
                """ps: [128, 512] psum f32 (4 heads). Writes normalized bf16
                rope output to out_bf [128, 4, 128]."""
                v3 = ps.rearrange("p (h d) -> p h d", h=4)
                ro = epipool.tile([P, 4, HD], f32, tag="ro", name="ro")
                cs = epipool.tile([P, 4, HD], f32, tag="cs", name="cs")
                sn = epipool.tile([P, 4, HD], f32, tag="sn", name="sn")
                nc.vector.tensor_tensor(cs[:, :, 0:64], v3[:, :, 0:64], c4, op=OP.mult)
                nc.vector.tensor_tensor(cs[:, :, 64:128], v3[:, :, 64:128], c4, op=OP.mult)
                nc.vector.tensor_tensor(sn[:, :, 0:64], v3[:, :, 0:64], s4, op=OP.mult)
                nc.vector.tensor_tensor(sn[:, :, 64:128], v3[:, :, 64:128], s4, op=OP.mult)
                nc.vector.tensor_tensor(ro[:, :, 0:64], cs[:, :, 0:64], sn[:, :, 64:128], op=OP.add)
                nc.vector.tensor_sub(ro[:, :, 64:128], cs[:, :, 64:128], sn[:, :, 0:64])
                ss = smallf.tile([P, 4], f32, tag="ss", name="ss")
                sq = epipool.tile([P, 4, HD], f32, tag="cs", name="sq")
                nc.vector.tensor_tensor(sq, ro, ro, op=OP.mult)
                nc.vector.reduce_sum(ss, sq, axis=AX.X)
                rms = smallf.tile([P, 4], f32, tag="rms", name="rms")
                nc.scalar.activation(rms, ss, AF.Sqrt, bias=eps_ap, scale=sqrt_scale)
                rinv = smallf.tile([P, 4], f32, tag="rms", name="rinv")
                nc.vector.reciprocal_approx_fast(rinv, rms)
                for hh in range(4):
                    nc.vector.tensor_scalar_mul(
                        out_bf[:, hh, :], ro[:, hh, :], rinv[:, hh:hh + 1]
                    )

            def pack_transpose(src_bf, dst):
                """src_bf [128, 4, 128] bf16 -> 4 PE transposes -> one copy to
                dst ([128, 4, 128] or [128, 512] view)."""
                ptr = psY.tile([P, 512], bf16, tag="Y", name="ptrq")
                for hh in range(4):
                    nc.tensor.transpose(
                        ptr[:, hh * P:(hh + 1) * P], src_bf[:, hh, :], ident
                    )
                if len(dst.ap) == 2:
                    nc.vector.tensor_copy(dst, ptr)
                else:
                    nc.vector.tensor_copy(dst, ptr.rearrange("p (s n) -> p s n", s=4))

            # ---------------- phase 1: K/V proj (own rows) + AllGather ------
            wslk = load_w_slab(wk, 0, "wk")
            ksb = bigpool.tile([P, NQT, 512], bf16, tag="ksb")
            for tt in range(NQT):
                ps = psS.tile([P, 512], f32, tag="S", name="psk")
                for kt in range(NCT):
                    nc.tensor.matmul(
                        ps,
                        xT[:, kt, tt * P:(tt + 1) * P],
                        wslk[:, kt, :],
                        start=(kt == 0),
                        stop=(kt == NCT - 1),
                    )
                khat = qhpool.tile([P, 4, HD], bf16, tag="qhat", name="khat")
                # fold attn scale into k's rms: 1/sqrt(ss + 128*eps)
                rope_rms(ps, cos4[tt], sin4[tt], khat, eps_k, 1.0)
                pack_transpose(khat, ksb[:, tt, :])
            nc.gpsimd.dma_start(
                out=cc_inK.rearrange("t p n -> p t n"), in_=ksb
            )
            nc.gpsimd.collective_compute(
                "AllGather",
                mybir.AluOpType.bypass,
                replica_groups=[[0, 1, 2, 3], [4, 5, 6, 7]],
                ins=[cc_inK[:]],
                outs=[cc_outK[:]],
            )
            # gathered K -> SBUF. cc_outK[r, tt] is absolute key chunk 4*tt+r.
            # kT free layout [kvh, k=(tt, r, ki)]
            for kvh in range(NKV):
                for r in range(4):
                    eng = nc.scalar if (r % 2 == 0) else nc.sync
                    eng.dma_start(
                        out=kT[:, kvh, :].rearrange(
                            "p (t f) -> p t f", t=4
                        )[:, :, r * P:(r + 1) * P],
                        in_=cc_outK[r, :, :, kvh * P:(kvh + 1) * P].rearrange(
                            "t p n -> p t n"
                        ),
                    )

            wslv = load_w_slab(wv, 0, "wv")
            vsb = bigpool.tile([P, NQT, 512], bf16, tag="vsb")
            for tt in range(NQT):
                psv = psS.tile([P, 512], f32, tag="S", name="psv")
                for kt in range(NCT):
                    nc.tensor.matmul(
                        psv,
                        xT[:, kt, tt * P:(tt + 1) * P],
                        wslv[:, kt, :],
                        start=(kt == 0),
                        stop=(kt == NCT - 1),
                    )
                nc.vector.tensor_copy(vsb[:, tt, :], psv)
            nc.gpsimd.dma_start(
                out=cc_inV.rearrange("t p n -> p t n"), in_=vsb
            )
            nc.gpsimd.collective_compute(
                "AllGather",
                mybir.AluOpType.bypass,
                replica_groups=[[0, 1, 2, 3], [4, 5, 6, 7]],
                ins=[cc_inV[:]],
                outs=[cc_outV[:]],
            )
            # vA[ki, ch=(tt, r), kvh, d]
            for kvh in range(NKV):
                for r in range(4):
                    eng = nc.scalar if (r % 2 == 0) else nc.sync
                    eng.dma_start(
                        out=vA[:, :, kvh, :].rearrange(
                            "p (t rr) n -> p t rr n", t=4
                        )[:, :, r, :],
                        in_=cc_outV[r, :, :, kvh * P:(kvh + 1) * P].rearrange(
                            "t p n -> p t n"
                        ),
                    )

            # prefetch first wo slabs; their DMAs run under later phases
            w3s = {0: None, 1: None}

            def load_wo_slab(s3):
                w3 = wopool.tile([P, NCT, 512], bf16, tag="wo3", name=f"wo{s3}")
                nc.sync.dma_start(
                    out=w3,
                    in_=wo[:, s3 * 512:s3 * 512 + 512].rearrange(
                        "(a p) n -> p a n", p=P
                    ),
                )
                return w3

            # ---------------- phase 2+3 interleaved: Q proj slab s, then ----
            # attention for heads 4s..4s+3.
            tail_state = []  # (yt_psum, den_psum, h)

            def emit_tail():
                if not tail_state:
                    return
                yt, den, h = tail_state.pop(0)
                rinv = smallf.tile([P, QROWS], f32, tag="rq", name="rqinv")
                nc.vector.reciprocal_approx_fast(rinv, den)
                nc.vector.tensor_tensor(yT[:, h, :], yt, rinv, op=OP.mult)

            def attn_head(h):
                kvh = h // (NH // NKV)
                yt = psY.tile([P, QROWS], f32, tag="Y", name="yt")
                den = psD.tile([P, QROWS], f32, tag="D", name="den")
                dq = []  # exp'd tiles awaiting PV (2-deep pipeline)
                ptsum = None
                for kt in range(NT):
                    # q-slot columns are stored high-slot-first, so the
                    # still-valid slots for key tile kt are columns [0, n)
                    n = QROWS - (kt // 4) * P
                    S = psS.tile([P, 512], f32, tag="S", name="Sb")
                    nc.tensor.matmul(
                        S[:, 0:n],
                        kT[:, kvh, kt * P:(kt + 1) * P],
                        qT[:, h, 0:n],
                        start=True,
                        stop=True,
                        skip_group_check=True,
                    )
                    if kt == 0 and tail_state:
                        emit_tail()
                    # attn scale already folded into k's rms normalization
                    pt = ptpool.tile([P, 512], bf16, tag="pt", name="pt")
                    nc.scalar.activation(pt[:, 0:n], S[:, 0:n], AF.Exp, scale=1.0)
                    # causal 0/1 mask on the diagonal slot (last 128 cols)
                    nc.vector.tensor_tensor(
                        pt[:, n - P:n], pt[:, n - P:n],
                        qmask[:, kt // 4, kt % 4, :], op=OP.mult,
                    )
                    # group-of-4 pre-sum for the denominator
                    if kt % 4 == 1:
                        prev = dq[-1][0]
                        ptsum = ptspool.tile([P, 512], bf16, tag="pts", name="pts")
                        nc.vector.tensor_tensor(
                            ptsum[:, 0:n], prev[:, 0:n], pt[:, 0:n], op=OP.add
                        )
                    elif kt % 4 in (2, 3):
                        nc.vector.tensor_tensor(
                            ptsum[:, 0:n], ptsum[:, 0:n], pt[:, 0:n], op=OP.add
                        )
                        if kt % 4 == 3:
                            nc.tensor.matmul(
                                den[:, 0:n], ones128, ptsum[:, 0:n],
                                start=(kt == 3), stop=(kt == NT - 1),
                                skip_group_check=True,
                            )
                    dq.append((pt, kt, n))
                    if len(dq) > 2:
                        ppt, pkt, pn = dq.pop(0)
                        nc.tensor.matmul(
                            yt[:, 0:pn], vA[:, pkt, kvh, :], ppt[:, 0:pn],
                            start=(pkt == 0), stop=(pkt == NT - 1),
                            skip_group_check=True,
                        )
                while dq:
                    ppt, pkt, pn = dq.pop(0)
                    nc.tensor.matmul(
                        yt[:, 0:pn], vA[:, pkt, kvh, :], ppt[:, 0:pn],
                        start=(pkt == 0), stop=(pkt == NT - 1),
                        skip_group_check=True,
                    )
                tail_state.append((yt, den, h))

            for s in range(4):
                wsl = load_w_slab(wq, s * 512, f"wq{s}")
                packs = []
                for tt in range(NQT):
                    ps = psS.tile([P, 512], f32, tag="S", name="psq")
                    for kt in range(NCT):
                        nc.tensor.matmul(
                            ps,
                            xT[:, kt, tt * P:(tt + 1) * P],
                            wsl[:, kt, :],
                            start=(kt == 0),
                            stop=(kt == NCT - 1),
                        )
                    qhat = qhpool.tile([P, 4, HD], bf16, tag="qhat", name="qhat")
                    rope_rms(ps, cos4[tt], sin4[tt], qhat, eps_q, 1.0 / HD)
                    pack_transpose(
                        qhat, qT[:, 4 * s:4 * s + 4, (3 - tt) * P:(4 - tt) * P]
                    )
                if s == 0:
                    w3s[0] = load_wo_slab(0)
                for h in range(4 * s, 4 * s + 4):
                    attn_head(h)
                if s == 0:
                    w3s[1] = load_wo_slab(1)
            emit_tail()

            # ---------------- phase 4: output projection ----------------
            for s3 in range(4):
                w3 = w3s.pop(s3)
                if s3 + 2 < 4:
                    w3s[s3 + 2] = load_wo_slab(s3 + 2)
                for qt in range(4):
                    ps = psS.tile([P, 512], f32, tag="S", name="ps3")
                    for ct in range(NCT):
                        nc.tensor.matmul(
                            ps,
                            yT[:, ct, (3 - qt) * P:(4 - qt) * P],
                            w3[:, ct, :],
                            start=(ct == 0),
                            stop=(ct == NCT - 1),
                        )
                    ot = outpool.tile([P, 512], f32, tag="ot", name="ot")
                    nc.vector.tensor_copy(ot, ps)
                    nc.sync.dma_start(
                        out=yo[qt * P:(qt + 1) * P, s3 * 512:(s3 + 1) * 512],
                        in_=ot,
                    )

    nc.compile()
    return nc


def _get_nc():
    if "nc" not in _CACHE:
        _CACHE["nc"] = _build()
    return _CACHE["nc"]


def _in_maps(x, cosr, sinr, wq, wk, wv, wo):
    import ml_dtypes

    bf = ml_dtypes.bfloat16
    wqb = wq.astype(bf)
    wkb = wk.astype(bf)
    wvb = wv.astype(bf)
    wob = wo.astype(bf)
    maps = []
    for core in range(8):
        b, g = core // 4, core % 4
        rows = _rows(g)
        xoT = np.ascontiguousarray(
            x[b][rows].T.astype(bf).reshape(NCT, P, QROWS)
        )
        maps.append({
            "xoT": xoT,
            "coso": np.ascontiguousarray(cosr[rows]),
            "sino": np.ascontiguousarray(sinr[rows]),
            "wq": wqb, "wk": wkb, "wv": wvb, "wo": wob,
            "qm": _qmask01_t(g).astype(bf),
        })
    return maps


def kernel(x, cos, sin, wq, wk, wv, wo):
    from concourse.bass_utils import run_bass_kernel_spmd

    x = np.ascontiguousarray(np.asarray(x, np.float32))
    cosr = np.ascontiguousarray(np.asarray(cos, np.float32).reshape(T, HD // 2))
    sinr = np.ascontiguousarray(np.asarray(sin, np.float32).reshape(T, HD // 2))
    wq = np.ascontiguousarray(np.asarray(wq, np.float32))
    wk = np.ascontiguousarray(np.asarray(wk, np.float32))
    wv = np.ascontiguousarray(np.asarray(wv, np.float32))
    wo = np.ascontiguousarray(np.asarray(wo, np.float32))

    nc = _get_nc()
    maps = _in_maps(x, cosr, sinr, wq, wk, wv, wo)
    _CACHE["in_maps"] = maps
    res = run_bass_kernel_spmd(nc, maps, list(range(8)))
    y = np.empty((B, T, C), np.float32)
    for core in range(8):
        b, g = core // 4, core % 4
        y[b][_rows(g)] = res.results[core]["yo"]
    return y


# revision 16
# speedup vs baseline: 1.0599x; 1.0599x over previous
"""Causal self-attention (GQA, rope, qk-rmsnorm) Trainium2 kernel, 8 NeuronCores.

Sharding: core = (b, g), b = core // 4 (batch), g = core % 4.
Each core handles query row-chunks {g, 4+g, 8+g, 12+g} (128 rows each) of its
batch: computes Q for those 512 rows, K/V for its OWN 512 rows only (the four
cores of a batch AllGather K/V), attention for all 16 heads over its
512 query rows, and its 512-row slice of the output projection.
Host gathers row slices. SPMD: all per-core variation comes via input shards.

Host pre-transposes x and pre-tiles all weights into partition-major
[128, 16, 512] tiles so every large DMA is contiguous per partition.

Slot c (c = 0..3) covers query chunk 4c+g with keys [0, 512*(c+1)) — uniform
across cores; causal masking inside the last 512 keys comes from a
host-provided additive mask shard (applied on the PE).
"""

import sys

if "/opt/trn_rl_repo" not in sys.path:
    sys.path.insert(0, "/opt/trn_rl_repo")

import numpy as np

B, T, C = 2, 2048, 2048
NH, NKV = 16, 4
HD = C // NH  # 128
P = 128
NT = T // P            # 16 token tiles per batch
NCT = C // P           # 16 contraction tiles
QROWS = 512            # own query rows per core
NQT = QROWS // P       # 4 own token tiles
EPS = float(np.finfo(np.float32).eps)
NEG = -1.0e9

_CACHE = {}


def _chunks(g):
    return [g, 4 + g, 8 + g, 12 + g]


def _rows(g):
    return np.concatenate([np.arange(ch * P, (ch + 1) * P) for ch in _chunks(g)])


def _qmask_t(g):
    """Additive mask, transposed layout: (slot c, sub s, k i, q j)."""
    m = np.zeros((4, 4, P, P), np.float32)
    for c in range(4):
        k0 = 512 * c
        r0 = (4 * c + g) * P
        k = k0 + np.arange(512)[:, None]          # (512, 1)
        q = r0 + np.arange(P)[None, :]            # (1, 128)
        m[c] = np.where(k <= q, 0.0, NEG).reshape(4, P, P)
    return m


def _build():
    import concourse.bacc as bacc
    import concourse.bass as bass
    import concourse.mybir as mybir
    import concourse.tile as tile
    from concourse.masks import make_identity

    f32 = mybir.dt.float32
    bf16 = mybir.dt.bfloat16
    fp8 = mybir.dt.float8e4
    AF = mybir.ActivationFunctionType
    OP = mybir.AluOpType
    AX = mybir.AxisListType
    DR = mybir.MatmulPerfMode.DoubleRow

    nc = bacc.Bacc("TRN2", target_bir_lowering=False, debug=False, num_devices=8)

    xoT = nc.dram_tensor("xoT", [P, NCT, QROWS], bf16, kind="ExternalInput").ap()
    coso = nc.dram_tensor("coso", [QROWS, HD // 2], f32, kind="ExternalInput").ap()
    sino = nc.dram_tensor("sino", [QROWS, HD // 2], f32, kind="ExternalInput").ap()
    # weights pre-tiled [slab, 128, 16, 512]
    wq = nc.dram_tensor("wq", [4, P, NCT, 512], bf16, kind="ExternalInput").ap()
    wk = nc.dram_tensor("wk", [P, NCT, 512], bf16, kind="ExternalInput").ap()
    wv = nc.dram_tensor("wv", [P, NCT, 512], bf16, kind="ExternalInput").ap()
    wo = nc.dram_tensor("wo", [4, P, NCT, 512], bf16, kind="ExternalInput").ap()
    qm = nc.dram_tensor("qm", [4, 4, P, P], bf16, kind="ExternalInput").ap()
    yo = nc.dram_tensor("yo", [QROWS, C], f32, kind="ExternalOutput").ap()

    def bcast4(ap2d):
        return bass.AP(
            tensor=ap2d.tensor,
            offset=ap2d.offset,
            ap=[ap2d.ap[0], [0, 4], ap2d.ap[1]],
        )

    with tile.TileContext(nc) as tc:
        with (
            tc.tile_pool(name="singles", bufs=1) as singles,
            tc.tile_pool(name="big", bufs=1) as bigpool,
            tc.tile_pool(name="wsl", bufs=2) as wslpool,
            tc.tile_pool(name="epi", bufs=2) as epipool,
            tc.tile_pool(name="qh", bufs=3) as qhpool,
            tc.tile_pool(name="wo3", bufs=2) as wopool,
            tc.tile_pool(name="pt", bufs=4) as ptpool,
            tc.tile_pool(name="pts", bufs=2) as ptspool,
            tc.tile_pool(name="smallf", bufs=2) as smallf,
            tc.tile_pool(name="outs", bufs=2) as outpool,
            tc.tile_pool(name="dram", bufs=1, space="DRAM") as drampool,
            tc.tile_pool(name="psS", bufs=3, space="PSUM") as psS,
            tc.tile_pool(name="psY", bufs=3, space="PSUM") as psY,
            tc.tile_pool(name="psD", bufs=2, space="PSUM") as psD,
        ):
            ident = singles.tile([P, P], bf16)
            make_identity(nc, ident)
            ones128 = singles.tile([P, P], bf16)
            nc.vector.memset(ones128, 1.0)
            eps_q = singles.tile([P, 1], f32)
            nc.vector.memset(eps_q, EPS)
            # k rms folds the attn 1/sqrt(HD): khat = ro/sqrt(ss + HD*eps)
            eps_k = singles.tile([P, 1], f32)
            nc.vector.memset(eps_k, HD * EPS)

            # persistent big SBUF tensors
            xT = bigpool.tile([P, NCT, QROWS], bf16, tag="xT")     # [c, ct, q]
            qT = bigpool.tile([P, NH, QROWS], bf16, tag="qT")      # [d, h, q]
            kT = bigpool.tile([P, NKV, T], bf16, tag="kT")         # [d, kvh, k]
            vA = bigpool.tile([P, NT, NKV, HD], bf16, tag="vA")    # [ktok, tt, kvh, d]
            yT = bigpool.tile([P, NCT, QROWS], bf16, tag="yT")     # [d, ct, q]
            qmask = singles.tile([P, 4, 4, P], bf16)               # [ki, c, sub, q]

            # loads: xT + wk race ahead on separate queues
            nc.gpsimd.dma_start(out=xT, in_=xoT)
            wslk = wslpool.tile([P, NCT, 512], bf16, tag="wsl", name="wk")
            nc.sync.dma_start(out=wslk, in_=wk)

            cos4 = [singles.tile([P, 4, 64], f32, name=f"cos4_{t}") for t in range(NQT)]
            sin4 = [singles.tile([P, 4, 64], f32, name=f"sin4_{t}") for t in range(NQT)]
            for t in range(NQT):
                nc.scalar.dma_start(out=cos4[t], in_=bcast4(coso[t * P:(t + 1) * P, :]))
                nc.scalar.dma_start(out=sin4[t], in_=bcast4(sino[t * P:(t + 1) * P, :]))
            nc.scalar.dma_start(out=qmask, in_=qm.rearrange("c s i j -> i c s j"))

            # collective bounce buffers (DRAM)
            cc_inK = drampool.tile([P, NQT, 512], bf16)
            cc_outK = drampool.tile([4, P, NQT, 512], bf16)
            cc_inV = drampool.tile([P, NQT, 512], bf16)
            cc_outV = drampool.tile([4, P, NQT, 512], bf16)

            def load_w_slab(w_ap, name):
                wsl = wslpool.tile([P, NCT, 512], bf16, tag="wsl", name=name)
                nc.sync.dma_start(out=wsl, in_=w_ap)
                return wsl

            def rope_rms(ps, c4, s4, out_bf, eps_ap, sqrt_scale):
                """ps: [128, 512] psum f32 (4 heads). Writes normalized bf16
                rope output to out_bf [128, 4, 128]."""
                v3 = ps.rearrange("p (h d) -> p h d", h=4)
                ro = epipool.tile([P, 4, HD], f32, tag="ro", name="ro")
                cs = epipool.tile([P, 4, HD], f32, tag="cs", name="cs")
                sn = epipool.tile([P, 4, HD], f32, tag="sn", name="sn")
                nc.vector.tensor_tensor(cs[:, :, 0:64], v3[:, :, 0:64], c4, op=OP.mult)
                nc.vector.tensor_tensor(cs[:, :, 64:128], v3[:, :, 64:128], c4, op=OP.mult)
                nc.vector.tensor_tensor(sn[:, :, 0:64], v3[:, :, 0:64], s4, op=OP.mult)
                nc.vector.tensor_tensor(sn[:, :, 64:128], v3[:, :, 64:128], s4, op=OP.mult)
                nc.vector.tensor_tensor(ro[:, :, 0:64], cs[:, :, 0:64], sn[:, :, 64:128], op=OP.add)
                nc.vector.tensor_sub(ro[:, :, 64:128], cs[:, :, 64:128], sn[:, :, 0:64])
                ss = smallf.tile([P, 4], f32, tag="ss", name="ss")
                sq = epipool.tile([P, 4, HD], f32, tag="cs", name="sq")
                nc.vector.tensor_tensor(sq, ro, ro, op=OP.mult)
                nc.vector.reduce_sum(ss, sq, axis=AX.X)
                rms = smallf.tile([P, 4], f32, tag="rms", name="rms")
                nc.scalar.activation(rms, ss, AF.Sqrt, bias=eps_ap, scale=sqrt_scale)
                rinv = smallf.tile([P, 4], f32, tag="rms", name="rinv")
                nc.vector.reciprocal_approx_fast(rinv, rms)
                for hh in range(4):
                    nc.vector.tensor_scalar_mul(
                        out_bf[:, hh, :], ro[:, hh, :], rinv[:, hh:hh + 1]
                    )

            def pack_transpose(src_bf, dst):
                ptr = psY.tile([P, 512], bf16, tag="Y", name="ptrq")
                for hh in range(4):
                    nc.tensor.transpose(
                        ptr[:, hh * P:(hh + 1) * P], src_bf[:, hh, :], ident
                    )
                if len(dst.ap) == 2:
                    nc.vector.tensor_copy(dst, ptr)
                else:
                    nc.vector.tensor_copy(dst, ptr.rearrange("p (s n) -> p s n", s=4))

            # ---------------- phase 1a: K proj (own rows) + AllGather -------
            ksb = bigpool.tile([P, NQT, 512], bf16, tag="ksb")
            for tt in range(NQT):
                ps = psS.tile([P, 512], f32, tag="S", name="psk")
                for kt in range(NCT):
                    nc.tensor.matmul(
                        ps,
                        xT[:, kt, tt * P:(tt + 1) * P],
                        wslk[:, kt, :],
                        start=(kt == 0),
                        stop=(kt == NCT - 1),
                    )
                khat = qhpool.tile([P, 4, HD], bf16, tag="qhat", name="khat")
                rope_rms(ps, cos4[tt], sin4[tt], khat, eps_k, 1.0)
                pack_transpose(khat, ksb[:, tt, :])
            nc.gpsimd.dma_start(out=cc_inK, in_=ksb)
            nc.gpsimd.collective_compute(
                "AllGather",
                mybir.AluOpType.bypass,
                replica_groups=[[0, 1, 2, 3], [4, 5, 6, 7]],
                ins=[cc_inK[:]],
                outs=[cc_outK[:]],
            )
            # gathered K -> SBUF; chunk of rank r, tile tt is absolute
            # key chunk 4*tt+r. kT free layout [kvh, k=(tt, r, ki)]
            for kvh in range(NKV):
                for r in range(4):
                    eng = nc.scalar if (r % 2 == 0) else nc.sync
                    eng.dma_start(
                        out=kT[:, kvh, :].rearrange(
                            "p (t f) -> p t f", t=4
                        )[:, :, r * P:(r + 1) * P],
                        in_=cc_outK[r, :, :, kvh * P:(kvh + 1) * P],
                    )

            # ---------------- phase 1b: V proj + AllGather ------------------
            wslv = load_w_slab(wv, "wv")
            vsb = bigpool.tile([P, NQT, 512], bf16, tag="vsb")
            for tt in range(NQT):
                psv = psS.tile([P, 512], f32, tag="S", name="psv")
                for kt in range(NCT):
                    nc.tensor.matmul(
                        psv,
                        xT[:, kt, tt * P:(tt + 1) * P],
                        wslv[:, kt, :],
                        start=(kt == 0),
                        stop=(kt == NCT - 1),
                    )
                nc.vector.tensor_copy(vsb[:, tt, :], psv)
            nc.gpsimd.dma_start(out=cc_inV, in_=vsb)
            nc.gpsimd.collective_compute(
                "AllGather",
                mybir.AluOpType.bypass,
                replica_groups=[[0, 1, 2, 3], [4, 5, 6, 7]],
                ins=[cc_inV[:]],
                outs=[cc_outV[:]],
            )
            # vA[ki, ch=(tt, r), kvh, d]
            for kvh in range(NKV):
                for r in range(4):
                    eng = nc.scalar if (r % 2 == 0) else nc.sync
                    eng.dma_start(
                        out=vA[:, :, kvh, :].rearrange(
                            "p (t rr) n -> p t rr n", t=4
                        )[:, :, r, :],
                        in_=cc_outV[r, :, :, kvh * P:(kvh + 1) * P],
                    )

            # ---------------- phase 2+3: Q slabs with look-behind attention -
            tail_state = []  # (yt_psum, den_psum, h)

            def emit_tail():
                if not tail_state:
                    return
                yt, den, h = tail_state.pop(0)
                rinv = smallf.tile([P, QROWS], f32, tag="rq", name="rqinv")
                nc.vector.reciprocal_approx_fast(rinv, den)
                nc.vector.tensor_tensor(yT[:, h, :], yt, rinv, op=OP.mult)

            def attn_head(h):
                kvh = h // (NH // NKV)
                yt = psY.tile([P, QROWS], f32, tag="Y", name="yt")
                den = psD.tile([P, QROWS], f32, tag="D", name="den")
                dq = []  # (pt, kt, n, ptsum|None) awaiting PV/den (2-deep)
                ptsum = None

                def emit(ent):
                    ppt, pkt, pn, psum_t = ent
                    nc.tensor.matmul(
                        yt[:, 0:pn], vA[:, pkt, kvh, :], ppt[:, 0:pn],
                        start=(pkt == 0), stop=(pkt == NT - 1),
                        skip_group_check=True,
                    )
                    if psum_t is not None:
                        nc.tensor.matmul(
                            den[:, 0:pn], ones128, psum_t[:, 0:pn],
                            start=(pkt == 3), stop=(pkt == NT - 1),
                            skip_group_check=True,
                        )

                for kt in range(NT):
                    n = QROWS - (kt // 4) * P
                    S = psS.tile([P, 512], f32, tag="S", name="Sb")
                    nc.tensor.matmul(
                        S[:, 0:n],
                        kT[:, kvh, kt * P:(kt + 1) * P],
                        qT[:, h, 0:n],
                        start=True,
                        stop=False,
                        skip_group_check=True,
                    )
                    nc.tensor.matmul(
                        S[:, n - P:n], ident, qmask[:, kt // 4, kt % 4, :],
                        start=False, stop=True, skip_group_check=True,
                    )
                    if kt == 0 and tail_state:
                        emit_tail()
                    pt = ptpool.tile([P, 512], bf16, tag="pt", name="pt")
                    nc.scalar.activation(pt[:, 0:n], S[:, 0:n], AF.Exp, scale=1.0)
                    ent_ptsum = None
                    if kt % 4 == 1:
                        prev = dq[-1][0]
                        ptsum = ptspool.tile([P, 512], bf16, tag="pts", name="pts")
                        nc.vector.tensor_tensor(
                            ptsum[:, 0:n], prev[:, 0:n], pt[:, 0:n], op=OP.add
                        )
                    elif kt % 4 in (2, 3):
                        nc.vector.tensor_tensor(
                            ptsum[:, 0:n], ptsum[:, 0:n], pt[:, 0:n], op=OP.add
                        )
                        if kt % 4 == 3:
                            ent_ptsum = ptsum
                    dq.append((pt, kt, n, ent_ptsum))
                    if len(dq) > 2:
                        emit(dq.pop(0))
                while dq:
                    emit(dq.pop(0))
                tail_state.append((yt, den, h))

            for s in range(4):
                wsl = load_w_slab(wq[s], f"wq{s}")
                for tt in range(NQT):
                    ps = psS.tile([P, 512], f32, tag="S", name="psq")
                    for kt in range(NCT):
                        nc.tensor.matmul(
                            ps,
                            xT[:, kt, tt * P:(tt + 1) * P],
                            wsl[:, kt, :],
                            start=(kt == 0),
                            stop=(kt == NCT - 1),
                        )
                    qhat = qhpool.tile([P, 4, HD], bf16, tag="qhat", name="qhat")
                    rope_rms(ps, cos4[tt], sin4[tt], qhat, eps_q, 1.0 / HD)
                    pack_transpose(
                        qhat, qT[:, 4 * s:4 * s + 4, (3 - tt) * P:(4 - tt) * P]
                    )
                if s >= 1:
                    for h in range(4 * (s - 1), 4 * (s - 1) + 4):
                        attn_head(h)
            for h in range(12, 16):
                attn_head(h)
            emit_tail()

            # ---------------- phase 4: output projection ----
            def load_wo_slab(s3):
                w3 = wopool.tile([P, NCT, 512], bf16, tag="wo3", name=f"wo{s3}")
                nc.sync.dma_start(out=w3, in_=wo[s3])
                return w3

            w3s = {0: load_wo_slab(0), 1: load_wo_slab(1)}
            for s3 in range(4):
                w3 = w3s.pop(s3)
                if s3 + 2 < 4:
                    w3s[s3 + 2] = load_wo_slab(s3 + 2)
                for qt in range(4):
                    ps = psS.tile([P, 512], f32, tag="S", name="ps3")
                    for ct in range(NCT):
                        nc.tensor.matmul(
                            ps,
                            yT[:, ct, (3 - qt) * P:(4 - qt) * P],
                            w3[:, ct, :],
                            start=(ct == 0),
                            stop=(ct == NCT - 1),
                        )
                    ot = outpool.tile([P, 512], f32, tag="ot", name="ot")
                    nc.vector.tensor_copy(ot, ps)
                    nc.sync.dma_start(
                        out=yo[qt * P:(qt + 1) * P, s3 * 512:(s3 + 1) * 512],
                        in_=ot,
                    )

    nc.compile()
    return nc


def _get_nc():
    if "nc" not in _CACHE:
        _CACHE["nc"] = _build()
    return _CACHE["nc"]


def _tile_w(w, bf):
    """[2048, ncols] -> [ncols//512, 128, 16, 512] partition-major tiles."""
    ncols = w.shape[1]
    return np.ascontiguousarray(
        w.reshape(NCT, P, ncols // 512, 512).transpose(2, 1, 0, 3).astype(bf)
    )


def _in_maps(x, cosr, sinr, wq, wk, wv, wo):
    import ml_dtypes

    bf = ml_dtypes.bfloat16
    wqb = _tile_w(wq, bf)                       # [4, 128, 16, 512]
    wkb = _tile_w(wk, bf)[0]                    # [128, 16, 512]
    wvb = _tile_w(wv, bf)[0]
    wob = _tile_w(wo, bf)                       # [4, 128, 16, 512]
    maps = []
    for core in range(8):
        b, g = core // 4, core % 4
        rows = _rows(g)
        xoT = np.ascontiguousarray(
            x[b][rows].T.astype(bf).reshape(NCT, P, QROWS).transpose(1, 0, 2)
        )
        maps.append({
            "xoT": xoT,
            "coso": np.ascontiguousarray(cosr[rows]),
            "sino": np.ascontiguousarray(sinr[rows]),
            "wq": wqb, "wk": wkb, "wv": wvb, "wo": wob,
            "qm": _qmask_t(g).astype(bf),
        })
    return maps


def kernel(x, cos, sin, wq, wk, wv, wo):
    from concourse.bass_utils import run_bass_kernel_spmd

    x = np.ascontiguousarray(np.asarray(x, np.float32))
    cosr = np.ascontiguousarray(np.asarray(cos, np.float32).reshape(T, HD // 2))
    sinr = np.ascontiguousarray(np.asarray(sin, np.float32).reshape(T, HD // 2))
    wq = np.ascontiguousarray(np.asarray(wq, np.float32))
    wk = np.ascontiguousarray(np.asarray(wk, np.float32))
    wv = np.ascontiguousarray(np.asarray(wv, np.float32))
    wo = np.ascontiguousarray(np.asarray(wo, np.float32))

    nc = _get_nc()
    maps = _in_maps(x, cosr, sinr, wq, wk, wv, wo)
    _CACHE["in_maps"] = maps
    res = run_bass_kernel_spmd(nc, maps, list(range(8)))
    y = np.empty((B, T, C), np.float32)
    for core in range(8):
        b, g = core // 4, core % 4
        y[b][_rows(g)] = res.results[core]["yo"]
    return y


# revision 18
# speedup vs baseline: 1.1005x; 1.0383x over previous
"""Causal self-attention (GQA, rope, qk-rmsnorm) Trainium2 kernel, 8 NeuronCores.

Sharding: core = (b, g), b = core // 4 (batch), g = core % 4.
Each core handles query row-chunks {g, 4+g, 8+g, 12+g} (128 rows each) of its
batch: computes Q for those 512 rows, K/V for its OWN 512 rows only (the four
cores of a batch AllGather K/V), attention for all 16 heads over its
512 query rows, and its 512-row slice of the output projection.
Host gathers row slices. SPMD: all per-core variation comes via input shards.

Host pre-transposes x and pre-tiles all weights into partition-major
[128, 16, 512] tiles so every large DMA is contiguous per partition.

Slot c (c = 0..3) covers query chunk 4c+g with keys [0, 512*(c+1)) — uniform
across cores; causal masking inside the last 512 keys comes from a
host-provided additive mask shard (applied on the PE).
"""

import sys

if "/opt/trn_rl_repo" not in sys.path:
    sys.path.insert(0, "/opt/trn_rl_repo")

import numpy as np

B, T, C = 2, 2048, 2048
NH, NKV = 16, 4
HD = C // NH  # 128
P = 128
NT = T // P            # 16 token tiles per batch
NCT = C // P           # 16 contraction tiles
QROWS = 512            # own query rows per core
NQT = QROWS // P       # 4 own token tiles
EPS = float(np.finfo(np.float32).eps)
NEG = -1.0e9

_CACHE = {}


def _chunks(g):
    return [g, 4 + g, 8 + g, 12 + g]


def _rows(g):
    return np.concatenate([np.arange(ch * P, (ch + 1) * P) for ch in _chunks(g)])


def _qmask_t(g):
    """Additive mask, transposed layout: (slot c, sub s, k i, q j)."""
    m = np.zeros((4, 4, P, P), np.float32)
    for c in range(4):
        k0 = 512 * c
        r0 = (4 * c + g) * P
        k = k0 + np.arange(512)[:, None]          # (512, 1)
        q = r0 + np.arange(P)[None, :]            # (1, 128)
        m[c] = np.where(k <= q, 0.0, NEG).reshape(4, P, P)
    return m


def _build():
    import concourse.bacc as bacc
    import concourse.bass as bass
    import concourse.mybir as mybir
    import concourse.tile as tile
    from concourse.masks import make_identity

    f32 = mybir.dt.float32
    bf16 = mybir.dt.bfloat16
    fp8 = mybir.dt.float8e4
    AF = mybir.ActivationFunctionType
    OP = mybir.AluOpType
    AX = mybir.AxisListType
    DR = mybir.MatmulPerfMode.DoubleRow

    nc = bacc.Bacc("TRN2", target_bir_lowering=False, debug=False, num_devices=8)

    xoT = nc.dram_tensor("xoT", [P, NCT, QROWS], bf16, kind="ExternalInput").ap()
    coso = nc.dram_tensor("coso", [QROWS, HD // 2], f32, kind="ExternalInput").ap()
    sino = nc.dram_tensor("sino", [QROWS, HD // 2], f32, kind="ExternalInput").ap()
    # weights pre-tiled [slab, 128, 16, 512]
    wq = nc.dram_tensor("wq", [4, P, NCT, 512], bf16, kind="ExternalInput").ap()
    wk = nc.dram_tensor("wk", [P, NCT, 512], bf16, kind="ExternalInput").ap()
    wv = nc.dram_tensor("wv", [P, NCT, 512], bf16, kind="ExternalInput").ap()
    wo = nc.dram_tensor("wo", [4, P, NCT, 512], bf16, kind="ExternalInput").ap()
    qm = nc.dram_tensor("qm", [4, 4, P, P], bf16, kind="ExternalInput").ap()
    yo = nc.dram_tensor("yo", [QROWS, C], f32, kind="ExternalOutput").ap()

    def bcast4(ap2d):
        return bass.AP(
            tensor=ap2d.tensor,
            offset=ap2d.offset,
            ap=[ap2d.ap[0], [0, 4], ap2d.ap[1]],
        )

    with tile.TileContext(nc) as tc:
        with (
            tc.tile_pool(name="singles", bufs=1) as singles,
            tc.tile_pool(name="big", bufs=1) as bigpool,
            tc.tile_pool(name="wsl", bufs=2) as wslpool,
            tc.tile_pool(name="epi", bufs=2) as epipool,
            tc.tile_pool(name="qh", bufs=3) as qhpool,
            tc.tile_pool(name="wo3", bufs=2) as wopool,
            tc.tile_pool(name="pt", bufs=4) as ptpool,
            tc.tile_pool(name="pts", bufs=2) as ptspool,
            tc.tile_pool(name="kvs", bufs=3) as kvspool,
            tc.tile_pool(name="smallf", bufs=2) as smallf,
            tc.tile_pool(name="outs", bufs=2) as outpool,
            tc.tile_pool(name="dram", bufs=1, space="DRAM") as drampool,
            tc.tile_pool(name="psS", bufs=3, space="PSUM") as psS,
            tc.tile_pool(name="psY", bufs=3, space="PSUM") as psY,
            tc.tile_pool(name="psD", bufs=2, space="PSUM") as psD,
        ):
            ident = singles.tile([P, P], bf16)
            make_identity(nc, ident)
            ones128 = singles.tile([P, P], bf16)
            nc.vector.memset(ones128, 1.0)
            eps_q = singles.tile([P, 1], f32)
            nc.vector.memset(eps_q, EPS)
            # k rms folds the attn 1/sqrt(HD): khat = ro/sqrt(ss + HD*eps)
            eps_k = singles.tile([P, 1], f32)
            nc.vector.memset(eps_k, HD * EPS)

            # persistent big SBUF tensors
            xT = bigpool.tile([P, NCT, QROWS], bf16, tag="xT")     # [c, ct, q]
            qT = bigpool.tile([P, NH, QROWS], bf16, tag="qT")      # [d, h, q]
            kT = bigpool.tile([P, NKV, T], bf16, tag="kT")         # [d, kvh, k]
            vA = bigpool.tile([P, NT, NKV, HD], bf16, tag="vA")    # [ktok, tt, kvh, d]
            yT = bigpool.tile([P, NCT, QROWS], bf16, tag="yT")     # [d, ct, q]
            qmask = singles.tile([P, 4, 4, P], bf16)               # [ki, c, sub, q]

            # loads: xT + wk race ahead on separate queues, finest first
            for ch in range(4):
                nc.gpsimd.dma_start(
                    out=xT[:, 4 * ch:4 * ch + 4, :], in_=xoT[:, 4 * ch:4 * ch + 4, :]
                )
            wslk = wslpool.tile([P, NCT, 512], bf16, tag="wsl", name="wk")
            for hf in range(2):
                nc.sync.dma_start(
                    out=wslk[:, 8 * hf:8 * hf + 8, :], in_=wk[:, 8 * hf:8 * hf + 8, :]
                )

            cos4 = [singles.tile([P, 4, 64], f32, name=f"cos4_{t}") for t in range(NQT)]
            sin4 = [singles.tile([P, 4, 64], f32, name=f"sin4_{t}") for t in range(NQT)]
            for t in range(NQT):
                nc.scalar.dma_start(out=cos4[t], in_=bcast4(coso[t * P:(t + 1) * P, :]))
                nc.scalar.dma_start(out=sin4[t], in_=bcast4(sino[t * P:(t + 1) * P, :]))

            # collective bounce buffers (DRAM)
            cc_inK = drampool.tile([P, NQT, 512], bf16)
            cc_outK = drampool.tile([4, P, NQT, 512], bf16)
            cc_inV = drampool.tile([P, NQT, 512], bf16)
            cc_outV = drampool.tile([4, P, NQT, 512], bf16)

            def load_w_slab(w_ap, name):
                wsl = wslpool.tile([P, NCT, 512], bf16, tag="wsl", name=name)
                nc.sync.dma_start(out=wsl, in_=w_ap)
                return wsl

            def rope_rms(ps, c4, s4, out_bf, eps_ap, sqrt_scale):
                """ps: [128, 512] psum f32 (4 heads). Writes normalized bf16
                rope output to out_bf [128, 4, 128]."""
                v3 = ps.rearrange("p (h d) -> p h d", h=4)
                ro = epipool.tile([P, 4, HD], f32, tag="ro", name="ro")
                cs = epipool.tile([P, 4, HD], f32, tag="cs", name="cs")
                sn = epipool.tile([P, 4, HD], f32, tag="sn", name="sn")
                nc.vector.tensor_tensor(cs[:, :, 0:64], v3[:, :, 0:64], c4, op=OP.mult)
                nc.vector.tensor_tensor(cs[:, :, 64:128], v3[:, :, 64:128], c4, op=OP.mult)
                nc.vector.tensor_tensor(sn[:, :, 0:64], v3[:, :, 0:64], s4, op=OP.mult)
                nc.vector.tensor_tensor(sn[:, :, 64:128], v3[:, :, 64:128], s4, op=OP.mult)
                nc.vector.tensor_tensor(ro[:, :, 0:64], cs[:, :, 0:64], sn[:, :, 64:128], op=OP.add)
                nc.vector.tensor_sub(ro[:, :, 64:128], cs[:, :, 64:128], sn[:, :, 0:64])
                ss = smallf.tile([P, 4], f32, tag="ss", name="ss")
                sq = epipool.tile([P, 4, HD], f32, tag="cs", name="sq")
                nc.vector.tensor_tensor(sq, ro, ro, op=OP.mult)
                nc.vector.reduce_sum(ss, sq, axis=AX.X)
                rms = smallf.tile([P, 4], f32, tag="rms", name="rms")
                nc.scalar.activation(rms, ss, AF.Sqrt, bias=eps_ap, scale=sqrt_scale)
                rinv = smallf.tile([P, 4], f32, tag="rms", name="rinv")
                nc.vector.reciprocal_approx_fast(rinv, rms)
                for hh in range(4):
                    nc.vector.tensor_scalar_mul(
                        out_bf[:, hh, :], ro[:, hh, :], rinv[:, hh:hh + 1]
                    )

            def pack_transpose(src_bf, dst):
                ptr = psY.tile([P, 512], bf16, tag="Y", name="ptrq")
                for hh in range(4):
                    nc.tensor.transpose(
                        ptr[:, hh * P:(hh + 1) * P], src_bf[:, hh, :], ident
                    )
                if len(dst.ap) == 2:
                    nc.vector.tensor_copy(dst, ptr)
                else:
                    nc.vector.tensor_copy(dst, ptr.rearrange("p (s n) -> p s n", s=4))

            # ---------------- phase 1a: K proj (own rows) + AllGather -------
            for tt in range(NQT):
                ps = psS.tile([P, 512], f32, tag="S", name="psk")
                for kt in range(NCT):
                    nc.tensor.matmul(
                        ps,
                        xT[:, kt, tt * P:(tt + 1) * P],
                        wslk[:, kt, :],
                        start=(kt == 0),
                        stop=(kt == NCT - 1),
                    )
                khat = qhpool.tile([P, 4, HD], bf16, tag="qhat", name="khat")
                rope_rms(ps, cos4[tt], sin4[tt], khat, eps_k, 1.0)
                ptr = psY.tile([P, 512], bf16, tag="Y", name="ptrk")
                for hh in range(4):
                    nc.tensor.transpose(
                        ptr[:, hh * P:(hh + 1) * P], khat[:, hh, :], ident
                    )
                ks = kvspool.tile([P, 512], bf16, tag="kvs", name="ks")
                nc.scalar.copy(ks, ptr)
                nc.gpsimd.dma_start(out=cc_inK[:, tt, :], in_=ks)
            nc.gpsimd.collective_compute(
                "AllGather",
                mybir.AluOpType.bypass,
                replica_groups=[[0, 1, 2, 3], [4, 5, 6, 7]],
                ins=[cc_inK[:]],
                outs=[cc_outK[:]],
            )
            # gathered K -> SBUF; chunk of rank r, tile tt is absolute
            # key chunk 4*tt+r. kT free layout [kvh, k=(tt, r, ki)]
            for kvh in range(NKV):
                for r in range(4):
                    eng = nc.scalar if (r % 2 == 0) else nc.sync
                    eng.dma_start(
                        out=kT[:, kvh, :].rearrange(
                            "p (t f) -> p t f", t=4
                        )[:, :, r * P:(r + 1) * P],
                        in_=cc_outK[r, :, :, kvh * P:(kvh + 1) * P],
                    )

            # ---------------- phase 1b: V proj + AllGather ------------------
            wslv = load_w_slab(wv, "wv")
            for tt in range(NQT):
                psv = psS.tile([P, 512], f32, tag="S", name="psv")
                for kt in range(NCT):
                    nc.tensor.matmul(
                        psv,
                        xT[:, kt, tt * P:(tt + 1) * P],
                        wslv[:, kt, :],
                        start=(kt == 0),
                        stop=(kt == NCT - 1),
                    )
                vs = kvspool.tile([P, 512], bf16, tag="kvs", name="vs")
                nc.scalar.copy(vs, psv)
                nc.gpsimd.dma_start(out=cc_inV[:, tt, :], in_=vs)
            nc.gpsimd.collective_compute(
                "AllGather",
                mybir.AluOpType.bypass,
                replica_groups=[[0, 1, 2, 3], [4, 5, 6, 7]],
                ins=[cc_inV[:]],
                outs=[cc_outV[:]],
            )
            nc.scalar.dma_start(out=qmask, in_=qm.rearrange("c s i j -> i c s j"))

            # vA[ki, ch=(tt, r), kvh, d]
            for kvh in range(NKV):
                for r in range(4):
                    eng = nc.scalar if (r % 2 == 0) else nc.sync
                    eng.dma_start(
                        out=vA[:, :, kvh, :].rearrange(
                            "p (t rr) n -> p t rr n", t=4
                        )[:, :, r, :],
                        in_=cc_outV[r, :, :, kvh * P:(kvh + 1) * P],
                    )

            # ---------------- phase 2+3: Q slabs with look-behind attention -
            tail_state = []  # (yt_psum, den_psum, h)

            def emit_tail():
                if not tail_state:
                    return
                yt, den, h = tail_state.pop(0)
                rinv = smallf.tile([P, QROWS], f32, tag="rq", name="rqinv")
                nc.vector.reciprocal_approx_fast(rinv, den)
                nc.vector.tensor_tensor(yT[:, h, :], yt, rinv, op=OP.mult)

            def attn_head(h):
                kvh = h // (NH // NKV)
                yt = psY.tile([P, QROWS], f32, tag="Y", name="yt")
                den = psD.tile([P, QROWS], f32, tag="D", name="den")
                dq = []  # (pt, kt, n, ptsum|None) awaiting PV/den (2-deep)
                ptsum = None

                def emit(ent):
                    ppt, pkt, pn, psum_t = ent
                    nc.tensor.matmul(
                        yt[:, 0:pn], vA[:, pkt, kvh, :], ppt[:, 0:pn],
                        start=(pkt == 0), stop=(pkt == NT - 1),
                        skip_group_check=True,
                    )
                    if psum_t is not None:
                        nc.tensor.matmul(
                            den[:, 0:pn], ones128, psum_t[:, 0:pn],
                            start=(pkt == 3), stop=(pkt == NT - 1),
                            skip_group_check=True,
                        )

                for kt in range(NT):
                    n = QROWS - (kt // 4) * P
                    S = psS.tile([P, 512], f32, tag="S", name="Sb")
                    nc.tensor.matmul(
                        S[:, 0:n],
                        kT[:, kvh, kt * P:(kt + 1) * P],
                        qT[:, h, 0:n],
                        start=True,
                        stop=False,
                        skip_group_check=True,
                    )
                    nc.tensor.matmul(
                        S[:, n - P:n], ident, qmask[:, kt // 4, kt % 4, :],
                        start=False, stop=True, skip_group_check=True,
                    )
                    if kt == 0 and tail_state:
                        emit_tail()
                    pt = ptpool.tile([P, 512], bf16, tag="pt", name="pt")
                    nc.scalar.activation(pt[:, 0:n], S[:, 0:n], AF.Exp, scale=1.0)
                    ent_ptsum = None
                    if kt % 4 == 1:
                        prev = dq[-1][0]
                        ptsum = ptspool.tile([P, 512], bf16, tag="pts", name="pts")
                        nc.vector.tensor_tensor(
                            ptsum[:, 0:n], prev[:, 0:n], pt[:, 0:n], op=OP.add
                        )
                    elif kt % 4 in (2, 3):
                        nc.vector.tensor_tensor(
                            ptsum[:, 0:n], ptsum[:, 0:n], pt[:, 0:n], op=OP.add
                        )
                        if kt % 4 == 3:
                            ent_ptsum = ptsum
                    dq.append((pt, kt, n, ent_ptsum))
                    if len(dq) > 2:
                        emit(dq.pop(0))
                while dq:
                    emit(dq.pop(0))
                tail_state.append((yt, den, h))

            for s in range(4):
                wsl = load_w_slab(wq[s], f"wq{s}")
                for tt in range(NQT):
                    ps = psS.tile([P, 512], f32, tag="S", name="psq")
                    for kt in range(NCT):
                        nc.tensor.matmul(
                            ps,
                            xT[:, kt, tt * P:(tt + 1) * P],
                            wsl[:, kt, :],
                            start=(kt == 0),
                            stop=(kt == NCT - 1),
                        )
                    qhat = qhpool.tile([P, 4, HD], bf16, tag="qhat", name="qhat")
                    rope_rms(ps, cos4[tt], sin4[tt], qhat, eps_q, 1.0 / HD)
                    pack_transpose(
                        qhat, qT[:, 4 * s:4 * s + 4, (3 - tt) * P:(4 - tt) * P]
                    )
                if s >= 1:
                    for h in range(4 * (s - 1), 4 * (s - 1) + 4):
                        attn_head(h)
            for h in range(12, 16):
                attn_head(h)
            emit_tail()

            # ---------------- phase 4: output projection ----
            def load_wo_slab(s3):
                w3 = wopool.tile([P, NCT, 512], bf16, tag="wo3", name=f"wo{s3}")
                nc.sync.dma_start(out=w3, in_=wo[s3])
                return w3

            w3s = {0: load_wo_slab(0), 1: load_wo_slab(1)}
            for s3 in range(4):
                w3 = w3s.pop(s3)
                if s3 + 2 < 4:
                    w3s[s3 + 2] = load_wo_slab(s3 + 2)
                for qt in range(4):
                    ps = psS.tile([P, 512], f32, tag="S", name="ps3")
                    for ct in range(NCT):
                        nc.tensor.matmul(
                            ps,
                            yT[:, ct, (3 - qt) * P:(4 - qt) * P],
                            w3[:, ct, :],
                            start=(ct == 0),
                            stop=(ct == NCT - 1),
                        )
                    ot = outpool.tile([P, 512], f32, tag="ot", name="ot")
                    nc.vector.tensor_copy(ot, ps)
                    nc.sync.dma_start(
                        out=yo[qt * P:(qt + 1) * P, s3 * 512:(s3 + 1) * 512],
                        in_=ot,
                    )

    nc.compile()
    return nc


def _get_nc():
    if "nc" not in _CACHE:
        _CACHE["nc"] = _build()
    return _CACHE["nc"]


def _tile_w(w, bf):
    """[2048, ncols] -> [ncols//512, 128, 16, 512] partition-major tiles."""
    ncols = w.shape[1]
    return np.ascontiguousarray(
        w.reshape(NCT, P, ncols // 512, 512).transpose(2, 1, 0, 3).astype(bf)
    )


def _in_maps(x, cosr, sinr, wq, wk, wv, wo):
    import ml_dtypes

    bf = ml_dtypes.bfloat16
    wqb = _tile_w(wq, bf)                       # [4, 128, 16, 512]
    wkb = _tile_w(wk, bf)[0]                    # [128, 16, 512]
    wvb = _tile_w(wv, bf)[0]
    wob = _tile_w(wo, bf)                       # [4, 128, 16, 512]
    maps = []
    for core in range(8):
        b, g = core // 4, core % 4
        rows = _rows(g)
        xoT = np.ascontiguousarray(
            x[b][rows].T.astype(bf).reshape(NCT, P, QROWS).transpose(1, 0, 2)
        )
        maps.append({
            "xoT": xoT,
            "coso": np.ascontiguousarray(cosr[rows]),
            "sino": np.ascontiguousarray(sinr[rows]),
            "wq": wqb, "wk": wkb, "wv": wvb, "wo": wob,
            "qm": _qmask_t(g).astype(bf),
        })
    return maps


def kernel(x, cos, sin, wq, wk, wv, wo):
    from concourse.bass_utils import run_bass_kernel_spmd

    x = np.ascontiguousarray(np.asarray(x, np.float32))
    cosr = np.ascontiguousarray(np.asarray(cos, np.float32).reshape(T, HD // 2))
    sinr = np.ascontiguousarray(np.asarray(sin, np.float32).reshape(T, HD // 2))
    wq = np.ascontiguousarray(np.asarray(wq, np.float32))
    wk = np.ascontiguousarray(np.asarray(wk, np.float32))
    wv = np.ascontiguousarray(np.asarray(wv, np.float32))
    wo = np.ascontiguousarray(np.asarray(wo, np.float32))

    nc = _get_nc()
    maps = _in_maps(x, cosr, sinr, wq, wk, wv, wo)
    _CACHE["in_maps"] = maps
    res = run_bass_kernel_spmd(nc, maps, list(range(8)))
    y = np.empty((B, T, C), np.float32)
    for core in range(8):
        b, g = core // 4, core % 4
        y[b][_rows(g)] = res.results[core]["yo"]
    return y


# revision 20
# speedup vs baseline: 1.1394x; 1.0353x over previous
"""Causal self-attention (GQA, rope, qk-rmsnorm) Trainium2 kernel, 8 NeuronCores.

Sharding: core = (b, g), b = core // 4 (batch), g = core % 4.
Each core handles query row-chunks {g, 4+g, 8+g, 12+g} (128 rows each) of its
batch: computes Q for those 512 rows, K/V for its OWN 512 rows only (the four
cores of a batch AllGather K/V), attention for all 16 heads over its
512 query rows, and its 512-row slice of the output projection.
Host gathers row slices. SPMD: all per-core variation comes via input shards.

Host pre-transposes x and pre-tiles all weights into partition-major
[128, 16, 512] tiles so every large DMA is contiguous per partition.

Slot c (c = 0..3) covers query chunk 4c+g with keys [0, 512*(c+1)) — uniform
across cores; causal masking inside the last 512 keys comes from a
host-provided additive mask shard (applied on the PE).
"""

import sys

if "/opt/trn_rl_repo" not in sys.path:
    sys.path.insert(0, "/opt/trn_rl_repo")

import numpy as np

B, T, C = 2, 2048, 2048
NH, NKV = 16, 4
HD = C // NH  # 128
P = 128
NT = T // P            # 16 token tiles per batch
NCT = C // P           # 16 contraction tiles
QROWS = 512            # own query rows per core
NQT = QROWS // P       # 4 own token tiles
EPS = float(np.finfo(np.float32).eps)
NEG = -1.0e9

_CACHE = {}


def _chunks(g):
    return [g, 4 + g, 8 + g, 12 + g]


def _rows(g):
    return np.concatenate([np.arange(ch * P, (ch + 1) * P) for ch in _chunks(g)])


def _qmask_t(g):
    """Additive mask, transposed layout: (slot c, sub s, k i, q j)."""
    m = np.zeros((4, 4, P, P), np.float32)
    for c in range(4):
        k0 = 512 * c
        r0 = (4 * c + g) * P
        k = k0 + np.arange(512)[:, None]          # (512, 1)
        q = r0 + np.arange(P)[None, :]            # (1, 128)
        m[c] = np.where(k <= q, 0.0, NEG).reshape(4, P, P)
    return m


def _build():
    import concourse.bacc as bacc
    import concourse.bass as bass
    import concourse.mybir as mybir
    import concourse.tile as tile
    from concourse.masks import make_identity

    f32 = mybir.dt.float32
    bf16 = mybir.dt.bfloat16
    fp8 = mybir.dt.float8e4
    AF = mybir.ActivationFunctionType
    OP = mybir.AluOpType
    AX = mybir.AxisListType
    DR = mybir.MatmulPerfMode.DoubleRow

    nc = bacc.Bacc("TRN2", target_bir_lowering=False, debug=False, num_devices=8)

    xoT = nc.dram_tensor("xoT", [P, NCT, QROWS], bf16, kind="ExternalInput").ap()
    cosb = nc.dram_tensor("cosb", [P, NQT, 4, HD // 2], f32, kind="ExternalInput").ap()
    sinb = nc.dram_tensor("sinb", [P, NQT, 4, HD // 2], f32, kind="ExternalInput").ap()
    # weights pre-tiled [slab, 128, 16, 512]
    wq = nc.dram_tensor("wq", [4, P, NCT, 512], bf16, kind="ExternalInput").ap()
    wk = nc.dram_tensor("wk", [P, NCT, 512], bf16, kind="ExternalInput").ap()
    wv = nc.dram_tensor("wv", [P, NCT, 512], bf16, kind="ExternalInput").ap()
    wo = nc.dram_tensor("wo", [4, P, NCT, 512], bf16, kind="ExternalInput").ap()
    qm = nc.dram_tensor("qm", [4, 4, P, P], bf16, kind="ExternalInput").ap()
    yo = nc.dram_tensor("yo", [QROWS, C], f32, kind="ExternalOutput").ap()

    with tile.TileContext(nc) as tc:
        with (
            tc.tile_pool(name="singles", bufs=1) as singles,
            tc.tile_pool(name="big", bufs=1) as bigpool,
            tc.tile_pool(name="wsl", bufs=2) as wslpool,
            tc.tile_pool(name="epi", bufs=2) as epipool,
            tc.tile_pool(name="qh", bufs=3) as qhpool,
            tc.tile_pool(name="wo3", bufs=2) as wopool,
            tc.tile_pool(name="pt", bufs=4) as ptpool,
            tc.tile_pool(name="pts", bufs=2) as ptspool,
            tc.tile_pool(name="kvs", bufs=3) as kvspool,
            tc.tile_pool(name="smallf", bufs=2) as smallf,
            tc.tile_pool(name="outs", bufs=2) as outpool,
            tc.tile_pool(name="dram", bufs=1, space="DRAM") as drampool,
            tc.tile_pool(name="psS", bufs=3, space="PSUM") as psS,
            tc.tile_pool(name="psY", bufs=3, space="PSUM") as psY,
            tc.tile_pool(name="psD", bufs=2, space="PSUM") as psD,
        ):
            ident = singles.tile([P, P], bf16)
            make_identity(nc, ident)
            ones128 = singles.tile([P, P], bf16)
            nc.vector.memset(ones128, 1.0)
            eps_q = singles.tile([P, 1], f32)
            nc.vector.memset(eps_q, EPS)
            # k rms folds the attn 1/sqrt(HD): khat = ro/sqrt(ss + HD*eps)
            eps_k = singles.tile([P, 1], f32)
            nc.vector.memset(eps_k, HD * EPS)

            # persistent big SBUF tensors
            xT = bigpool.tile([P, NCT, QROWS], bf16, tag="xT")     # [c, ct, q]
            qT = bigpool.tile([P, NH, QROWS], bf16, tag="qT")      # [d, h, q]
            kT = bigpool.tile([P, NKV, T], bf16, tag="kT")         # [d, kvh, k]
            vA = bigpool.tile([P, NT, NKV, HD], bf16, tag="vA")    # [ktok, tt, kvh, d]
            yT = bigpool.tile([P, NCT, QROWS], bf16, tag="yT")     # [d, ct, q]
            qmask = singles.tile([P, 4, 4, P], bf16)               # [ki, c, sub, q]

            # loads: xT + wk race ahead on separate queues, finest first
            for ch in range(4):
                nc.gpsimd.dma_start(
                    out=xT[:, 4 * ch:4 * ch + 4, :], in_=xoT[:, 4 * ch:4 * ch + 4, :]
                )
            wslk = wslpool.tile([P, NCT, 512], bf16, tag="wsl", name="wk")
            for hf in range(2):
                nc.sync.dma_start(
                    out=wslk[:, 8 * hf:8 * hf + 8, :], in_=wk[:, 8 * hf:8 * hf + 8, :]
                )

            cosT = singles.tile([P, NQT, 4, 64], f32, name="cosT")
            sinT = singles.tile([P, NQT, 4, 64], f32, name="sinT")
            nc.scalar.dma_start(out=cosT, in_=cosb)
            nc.scalar.dma_start(out=sinT, in_=sinb)
            cos4 = [cosT[:, t, :, :] for t in range(NQT)]
            sin4 = [sinT[:, t, :, :] for t in range(NQT)]

            # collective bounce buffers (DRAM)
            cc_inK = drampool.tile([P, NQT, 512], bf16)
            cc_outK = drampool.tile([4, P, NQT, 512], bf16)
            cc_inV = drampool.tile([P, NQT, 512], bf16)
            cc_outV = drampool.tile([4, P, NQT, 512], bf16)

            def load_w_slab(w_ap, name):
                wsl = wslpool.tile([P, NCT, 512], bf16, tag="wsl", name=name)
                nc.sync.dma_start(out=wsl, in_=w_ap)
                return wsl

            def rope_rms(ps, c4, s4, out_bf, eps_ap, sqrt_scale):
                """ps: [128, 512] psum f32 (4 heads). Writes normalized bf16
                rope output to out_bf [128, 4, 128]."""
                v3 = ps.rearrange("p (h d) -> p h d", h=4)
                ro = epipool.tile([P, 4, HD], f32, tag="ro", name="ro")
                cs = epipool.tile([P, 4, HD], f32, tag="cs", name="cs")
                sn = epipool.tile([P, 4, HD], f32, tag="sn", name="sn")
                nc.vector.tensor_tensor(cs[:, :, 0:64], v3[:, :, 0:64], c4, op=OP.mult)
                nc.vector.tensor_tensor(cs[:, :, 64:128], v3[:, :, 64:128], c4, op=OP.mult)
                nc.vector.tensor_tensor(sn[:, :, 0:64], v3[:, :, 0:64], s4, op=OP.mult)
                nc.vector.tensor_tensor(sn[:, :, 64:128], v3[:, :, 64:128], s4, op=OP.mult)
                nc.vector.tensor_tensor(ro[:, :, 0:64], cs[:, :, 0:64], sn[:, :, 64:128], op=OP.add)
                nc.vector.tensor_sub(ro[:, :, 64:128], cs[:, :, 64:128], sn[:, :, 0:64])
                ss = smallf.tile([P, 4], f32, tag="ss", name="ss")
                sq = epipool.tile([P, 4, HD], f32, tag="cs", name="sq")
                nc.vector.tensor_tensor(sq, ro, ro, op=OP.mult)
                nc.vector.reduce_sum(ss, sq, axis=AX.X)
                rms = smallf.tile([P, 4], f32, tag="rms", name="rms")
                nc.scalar.activation(rms, ss, AF.Sqrt, bias=eps_ap, scale=sqrt_scale)
                rinv = smallf.tile([P, 4], f32, tag="rms", name="rinv")
                nc.vector.reciprocal_approx_fast(rinv, rms)
                for hh in range(4):
                    nc.vector.tensor_scalar_mul(
                        out_bf[:, hh, :], ro[:, hh, :], rinv[:, hh:hh + 1]
                    )

            def pack_transpose(src_bf, dst):
                ptr = psY.tile([P, 512], bf16, tag="Y", name="ptrq")
                for hh in range(4):
                    nc.tensor.transpose(
                        ptr[:, hh * P:(hh + 1) * P], src_bf[:, hh, :], ident
                    )
                if len(dst.ap) == 2:
                    nc.vector.tensor_copy(dst, ptr)
                else:
                    nc.vector.tensor_copy(dst, ptr.rearrange("p (s n) -> p s n", s=4))

            # ---------------- phase 1a: K proj (own rows) + AllGather -------
            pending = []

            def drain_pending(keep=0):
                while len(pending) > keep:
                    pending.pop(0)()

            def pack_k(khat, tt):
                ptr = psY.tile([P, 512], bf16, tag="Y", name="ptrk")
                for hh in range(4):
                    nc.tensor.transpose(
                        ptr[:, hh * P:(hh + 1) * P], khat[:, hh, :], ident
                    )
                ks = kvspool.tile([P, 512], bf16, tag="kvs", name="ks")
                nc.scalar.copy(ks, ptr)
                nc.gpsimd.dma_start(out=cc_inK[:, tt, :], in_=ks)

            for tt in range(NQT):
                ps = psS.tile([P, 512], f32, tag="S", name="psk")
                for kt in range(NCT):
                    nc.tensor.matmul(
                        ps,
                        xT[:, kt, tt * P:(tt + 1) * P],
                        wslk[:, kt, :],
                        start=(kt == 0),
                        stop=(kt == NCT - 1),
                    )
                khat = qhpool.tile([P, 4, HD], bf16, tag="qhat", name="khat")
                rope_rms(ps, cos4[tt], sin4[tt], khat, eps_k, 1.0)
                drain_pending(1)
                pending.append(lambda khat=khat, tt=tt: pack_k(khat, tt))
            drain_pending()
            nc.gpsimd.collective_compute(
                "AllGather",
                mybir.AluOpType.bypass,
                replica_groups=[[0, 1, 2, 3], [4, 5, 6, 7]],
                ins=[cc_inK[:]],
                outs=[cc_outK[:]],
            )
            # gathered K -> SBUF; chunk of rank r, tile tt is absolute
            # key chunk 4*tt+r. kT free layout [kvh, k=(tt, r, ki)]
            for kvh in range(NKV):
                for r in range(4):
                    eng = nc.scalar if (r % 2 == 0) else nc.sync
                    eng.dma_start(
                        out=kT[:, kvh, :].rearrange(
                            "p (t f) -> p t f", t=4
                        )[:, :, r * P:(r + 1) * P],
                        in_=cc_outK[r, :, :, kvh * P:(kvh + 1) * P],
                    )

            # ---------------- phase 1b: V proj + AllGather ------------------
            wslv = load_w_slab(wv, "wv")
            for tt in range(NQT):
                psv = psS.tile([P, 512], f32, tag="S", name="psv")
                for kt in range(NCT):
                    nc.tensor.matmul(
                        psv,
                        xT[:, kt, tt * P:(tt + 1) * P],
                        wslv[:, kt, :],
                        start=(kt == 0),
                        stop=(kt == NCT - 1),
                    )
                vs = kvspool.tile([P, 512], bf16, tag="kvs", name="vs")
                nc.scalar.copy(vs, psv)
                nc.gpsimd.dma_start(out=cc_inV[:, tt, :], in_=vs)
            nc.gpsimd.collective_compute(
                "AllGather",
                mybir.AluOpType.bypass,
                replica_groups=[[0, 1, 2, 3], [4, 5, 6, 7]],
                ins=[cc_inV[:]],
                outs=[cc_outV[:]],
            )
            nc.scalar.dma_start(out=qmask, in_=qm.rearrange("c s i j -> i c s j"))

            # vA[ki, ch=(tt, r), kvh, d]
            for kvh in range(NKV):
                for r in range(4):
                    eng = nc.scalar if (r % 2 == 0) else nc.sync
                    eng.dma_start(
                        out=vA[:, :, kvh, :].rearrange(
                            "p (t rr) n -> p t rr n", t=4
                        )[:, :, r, :],
                        in_=cc_outV[r, :, :, kvh * P:(kvh + 1) * P],
                    )

            # ---------------- phase 2+3: Q slabs with look-behind attention -
            tail_state = []  # (yt_psum, den_psum, h)

            def emit_tail():
                if not tail_state:
                    return
                yt, den, h = tail_state.pop(0)
                rinv = smallf.tile([P, QROWS], f32, tag="rq", name="rqinv")
                nc.vector.reciprocal_approx_fast(rinv, den)
                nc.vector.tensor_tensor(yT[:, h, :], yt, rinv, op=OP.mult)

            def attn_head(h):
                kvh = h // (NH // NKV)
                yt = psY.tile([P, QROWS], f32, tag="Y", name="yt")
                den = psD.tile([P, QROWS], f32, tag="D", name="den")
                dq = []  # (pt, kt, n, ptsum|None) awaiting PV/den (2-deep)
                ptsum = None

                def emit(ent):
                    ppt, pkt, pn, psum_t = ent
                    nc.tensor.matmul(
                        yt[:, 0:pn], vA[:, pkt, kvh, :], ppt[:, 0:pn],
                        start=(pkt == 0), stop=(pkt == NT - 1),
                        skip_group_check=True,
                    )
                    if psum_t is not None:
                        nc.tensor.matmul(
                            den[:, 0:pn], ones128, psum_t[:, 0:pn],
                            start=(pkt == 3), stop=(pkt == NT - 1),
                            skip_group_check=True,
                        )

                for kt in range(NT):
                    n = QROWS - (kt // 4) * P
                    S = psS.tile([P, 512], f32, tag="S", name="Sb")
                    nc.tensor.matmul(
                        S[:, 0:n],
                        kT[:, kvh, kt * P:(kt + 1) * P],
                        qT[:, h, 0:n],
                        start=True,
                        stop=False,
                        skip_group_check=True,
                    )
                    nc.tensor.matmul(
                        S[:, n - P:n], ident, qmask[:, kt // 4, kt % 4, :],
                        start=False, stop=True, skip_group_check=True,
                    )
                    if kt == 0 and tail_state:
                        emit_tail()
                    pt = ptpool.tile([P, 512], bf16, tag="pt", name="pt")
                    nc.scalar.activation(pt[:, 0:n], S[:, 0:n], AF.Exp, scale=1.0)
                    ent_ptsum = None
                    if kt % 4 == 1:
                        prev = dq[-1][0]
                        ptsum = ptspool.tile([P, 512], bf16, tag="pts", name="pts")
                        nc.vector.tensor_tensor(
                            ptsum[:, 0:n], prev[:, 0:n], pt[:, 0:n], op=OP.add
                        )
                    elif kt % 4 in (2, 3):
                        nc.vector.tensor_tensor(
                            ptsum[:, 0:n], ptsum[:, 0:n], pt[:, 0:n], op=OP.add
                        )
                        if kt % 4 == 3:
                            ent_ptsum = ptsum
                    dq.append((pt, kt, n, ent_ptsum))
                    if len(dq) > 2:
                        emit(dq.pop(0))
                while dq:
                    emit(dq.pop(0))
                tail_state.append((yt, den, h))

            for s in range(4):
                wsl = load_w_slab(wq[s], f"wq{s}")
                for tt in range(NQT):
                    ps = psS.tile([P, 512], f32, tag="S", name="psq")
                    for kt in range(NCT):
                        nc.tensor.matmul(
                            ps,
                            xT[:, kt, tt * P:(tt + 1) * P],
                            wsl[:, kt, :],
                            start=(kt == 0),
                            stop=(kt == NCT - 1),
                        )
                    qhat = qhpool.tile([P, 4, HD], bf16, tag="qhat", name="qhat")
                    rope_rms(ps, cos4[tt], sin4[tt], qhat, eps_q, 1.0 / HD)
                    drain_pending(1)
                    pending.append(
                        lambda qhat=qhat, s=s, tt=tt: pack_transpose(
                            qhat, qT[:, 4 * s:4 * s + 4, (3 - tt) * P:(4 - tt) * P]
                        )
                    )
            drain_pending()
            for h in range(NH):
                attn_head(h)
            emit_tail()

            # ---------------- phase 4: output projection ----
            def load_wo_slab(s3):
                w3 = wopool.tile([P, NCT, 512], bf16, tag="wo3", name=f"wo{s3}")
                nc.sync.dma_start(out=w3, in_=wo[s3])
                return w3

            w3s = {0: load_wo_slab(0), 1: load_wo_slab(1)}
            for s3 in range(4):
                w3 = w3s.pop(s3)
                if s3 + 2 < 4:
                    w3s[s3 + 2] = load_wo_slab(s3 + 2)
                for qt in range(4):
                    ps = psS.tile([P, 512], f32, tag="S", name="ps3")
                    for ct in range(NCT):
                        nc.tensor.matmul(
                            ps,
                            yT[:, ct, (3 - qt) * P:(4 - qt) * P],
                            w3[:, ct, :],
                            start=(ct == 0),
                            stop=(ct == NCT - 1),
                        )
                    ot = outpool.tile([P, 512], f32, tag="ot", name="ot")
                    nc.vector.tensor_copy(ot, ps)
                    nc.sync.dma_start(
                        out=yo[qt * P:(qt + 1) * P, s3 * 512:(s3 + 1) * 512],
                        in_=ot,
                    )

    nc.compile()
    return nc


def _get_nc():
    if "nc" not in _CACHE:
        _CACHE["nc"] = _build()
    return _CACHE["nc"]


def _tile_w(w, bf):
    """[2048, ncols] -> [ncols//512, 128, 16, 512] partition-major tiles."""
    ncols = w.shape[1]
    return np.ascontiguousarray(
        w.reshape(NCT, P, ncols // 512, 512).transpose(2, 1, 0, 3).astype(bf)
    )


def _in_maps(x, cosr, sinr, wq, wk, wv, wo):
    import ml_dtypes

    bf = ml_dtypes.bfloat16
    wqb = _tile_w(wq, bf)                       # [4, 128, 16, 512]
    wkb = _tile_w(wk, bf)[0]                    # [128, 16, 512]
    wvb = _tile_w(wv, bf)[0]
    wob = _tile_w(wo, bf)                       # [4, 128, 16, 512]
    maps = []
    for core in range(8):
        b, g = core // 4, core % 4
        rows = _rows(g)
        xoT = np.ascontiguousarray(
            x[b][rows].T.astype(bf).reshape(NCT, P, QROWS).transpose(1, 0, 2)
        )
        def _bc(a):
            t = a[rows].reshape(NQT, P, HD // 2)
            t = np.broadcast_to(t[:, :, None, :], (NQT, P, 4, HD // 2))
            return np.ascontiguousarray(t.transpose(1, 0, 2, 3), dtype=np.float32)
        maps.append({
            "xoT": xoT,
            "cosb": _bc(cosr),
            "sinb": _bc(sinr),
            "wq": wqb, "wk": wkb, "wv": wvb, "wo": wob,
            "qm": _qmask_t(g).astype(bf),
        })
    return maps


def kernel(x, cos, sin, wq, wk, wv, wo):
    from concourse.bass_utils import run_bass_kernel_spmd

    x = np.ascontiguousarray(np.asarray(x, np.float32))
    cosr = np.ascontiguousarray(np.asarray(cos, np.float32).reshape(T, HD // 2))
    sinr = np.ascontiguousarray(np.asarray(sin, np.float32).reshape(T, HD // 2))
    wq = np.ascontiguousarray(np.asarray(wq, np.float32))
    wk = np.ascontiguousarray(np.asarray(wk, np.float32))
    wv = np.ascontiguousarray(np.asarray(wv, np.float32))
    wo = np.ascontiguousarray(np.asarray(wo, np.float32))

    nc = _get_nc()
    maps = _in_maps(x, cosr, sinr, wq, wk, wv, wo)
    _CACHE["in_maps"] = maps
    res = run_bass_kernel_spmd(nc, maps, list(range(8)))
    y = np.empty((B, T, C), np.float32)
    for core in range(8):
        b, g = core // 4, core % 4
        y[b][_rows(g)] = res.results[core]["yo"]
    return y
